# revision 28
# baseline (speedup 1.0000x reference)
"""CARAFE kernel for Trainium2 (8 NeuronCores, data+H-spatial sharded).

Full inputs in, full output out. Inside: host pre-shards X into per-core
zero-padded slabs (batch x H-quarter), one identical Bass/Tile program runs
SPMD on 8 cores, host gathers the output slabs.

v3: PE warmup chain (probe the cold-clock throttle), split/reordered input
DMA so conv1 starts early, conv2 emitted in 8-row halves (N=512 matmuls),
output laid out y-major for larger DMA chunks, edge-only memsets.
"""

import numpy as np

import concourse.bass as bass
import concourse.mybir as mybir
import concourse.tile as tile
from concourse import bacc, bass_utils

F32 = mybir.dt.float32
F16 = mybir.dt.float16
I16 = mybir.dt.int16
AF = mybir.ActivationFunctionType

SCALE = 2
K_UP = 5
K_ENC = 3
EPS = 1e-5
B, C, H, W = 2, 256, 64, 64
MID = 64
ENC = 100  # (SCALE*K_UP)**2
HQ = H // 4          # 16 input rows per core
RS = HQ + 4          # 20 slab rows (halo 2 each side)
XW = W + 4           # 68 padded width
WTW = W + 8          # 72 padded width for weight rows (phase-2 shifts)

WARMUP = 36          # dummy N=128 matmuls to try to unthrottle the PE clock

_CACHE = {}


def _channel_perm():
    """New enc-channel order: c' = j*20 + i*4 + py*2 + px.
    Original torch order: c = (i*K_UP + j)*4 + py*2 + px."""
    perm = np.zeros(ENC, dtype=np.int64)
    for j in range(K_UP):
        for i in range(K_UP):
            for py in range(SCALE):
                for px in range(SCALE):
                    newc = ((j * K_UP + i) * SCALE + py) * SCALE + px
                    oldc = ((i * K_UP + j) * SCALE + py) * SCALE + px
                    perm[newc] = oldc
    return perm


def _build_program_v3():
    """Banded-matmul reassembly. Per output row-pair Y: 5 shifted PE
    transposes of the normalized softmax weights produce
    V[x'-partition, (j,i,py,px)] (each transpose uses an identity SLICE so
    only its own 20-channel j-group is emitted); a gpsimd local_scatter
    places V into a sparse band matrix band[x', i*256 + py*128 + 2*(x'-j)
    + px]; then 5 fp16 matmuls against the host-pre-transposed X slab
    contract x' and accumulate over the i taps in PSUM."""
    nc = bacc.Bacc("TRN2", target_bir_lowering=False, debug=False)

    xslab = nc.dram_tensor("xslab", [128, 2, RS, XW], F16, kind="ExternalInput").ap()
    xslabT = nc.dram_tensor("xslabT", [XW, 2, RS, 128], F16, kind="ExternalInput").ap()
    # fp16 constants packed into blobs. blobA = conv1 weights (cols 0:128 of
    # the old blob16); blobB cols: [0:300) wencT2a (dx*100+oc), [300:600)
    # wencT2b, [600:604) sel4T, [604:704) selbT, [704:832) eye128
    blobA = nc.dram_tensor("blobA", [128, 128], F16, kind="ExternalInput").ap()
    blobB = nc.dram_tensor("blobB", [128, 832], F16, kind="ExternalInput").ap()
    # blob32 cols: 0 s1, 1 t1 (rows :64); 2 s2, 3 t2 (rows :100)
    blob32 = nc.dram_tensor("blob32", [128, 4], F32, kind="ExternalInput").ap()
    scatidx = nc.dram_tensor("scatidx", [80, ENC], I16, kind="ExternalInput").ap()
    # output y-major so per-partition DMA chunks are (2*2W) contiguous
    out = nc.dram_tensor("out", [128, 2 * HQ, 2, 2 * W], F16, kind="ExternalOutput").ap()

    with tile.TileContext(nc) as tc:
        with (
            tc.tile_pool(name="consts", bufs=1) as consts,
            tc.tile_pool(name="xpool", bufs=1) as xpool,
            tc.tile_pool(name="comp", bufs=1) as comppool,
            tc.tile_pool(name="wt", bufs=1) as wtpool,
            tc.tile_pool(name="vsb", bufs=4) as vpool,
            tc.tile_pool(name="band", bufs=17) as bandpool,
            tc.tile_pool(name="work", bufs=2) as work,
            tc.tile_pool(name="psout", bufs=5, space="PSUM") as psout,
            tc.tile_pool(name="psmisc", bufs=2, space="PSUM") as psmisc,
            tc.tile_pool(name="pstr", bufs=1, space="PSUM") as pstr,
        ):
            # ---- input DMAs, in consumption order, split across the two
            # HWDGE rings (SP + Activation) so they land in parallel ----
            cbA = consts.tile([128, 128], F16)
            nc.sync.dma_start(cbA[:], blobA)
            cb32 = consts.tile([128, 4], F32)
            nc.sync.dma_start(cb32[:], blob32)
            xA_sb = xpool.tile([128, 2, 13, XW], F16)
            # conv1 tile 0 needs slab rows 1..6 of both c-blocks: ship
            # those rows first so the first matmul can start sooner.
            nc.sync.dma_start(xA_sb[:, 0, 0:7], xslab[:, 0, 0:7, :])
            nc.scalar.dma_start(xA_sb[:, 1, 0:7], xslab[:, 1, 0:7, :])
            nc.sync.dma_start(xA_sb[:, 0, 7:13], xslab[:, 0, 7:13, :])
            nc.scalar.dma_start(xA_sb[:, 1, 7:13], xslab[:, 1, 7:13, :])
            cbB = consts.tile([128, 832], F16)
            nc.scalar.dma_start(cbB[:], blobB)
            xB_sb = xpool.tile([128, 2, RS - 13, XW], F16)
            nc.sync.dma_start(xB_sb[:, 0], xslab[:, 0, 13:RS, :])
            nc.scalar.dma_start(xB_sb[:, 1], xslab[:, 1, 13:RS, :])
            sidx_sb = consts.tile([80, ENC], I16)
            nc.scalar.dma_start(sidx_sb[:], scatidx)
            xT_sb = xpool.tile([XW, 2, RS, 128], F16)
            nc.sync.dma_start(xT_sb[:, 0], xslabT[:, 0])
            nc.scalar.dma_start(xT_sb[:, 1], xslabT[:, 1])
            out_sb = xpool.tile([128, 2 * HQ, 2, 2 * W], F16)

            # ---- conv1 + bn1 + relu -> comp [128, 18, 68] fp16 ----
            # rows 0:64 hold comp[r]; rows 64:128 hold comp[r+1] so the
            # enc conv can contract two dy taps per K=128 matmul.
            comp_sb = comppool.tile([128, RS - 2, XW], F16)
            # only the column halo (0:2, 66:68) is never written
            nc.vector.memset(comp_sb[:, :, 0:2], 0.0)
            nc.vector.memset(comp_sb[:, :, 2 + W:XW], 0.0)
            for rt in range(3):
                r0 = rt * 6
                xt, xoff = (xA_sb, 0) if rt < 2 else (xB_sb, 13)
                ps = psmisc.tile([128, 512], F32, tag="ps")
                for cb in range(2):
                    nc.tensor.matmul(
                        ps[:MID, :6 * W],
                        cbA[:, cb * MID:cb * MID + MID],
                        xt[:, cb, 1 + r0 - xoff:1 + r0 - xoff + 6, 2:2 + W],
                        start=(cb == 0),
                        stop=(cb == 1),
                    )
                nc.scalar.activation(
                    comp_sb[:MID, r0:r0 + 6, 2:2 + W],
                    ps[:MID, :6 * W].rearrange("p (r x) -> p r x", r=6),
                    AF.Relu, bias=cb32[:MID, 1:2], scale=cb32[:MID, 0:1],
                )
                lo = 1 if rt == 0 else 0
                nc.scalar.activation(
                    comp_sb[MID:128, r0 - 1 + lo:r0 + 5, 2:2 + W],
                    ps[:MID, :6 * W].rearrange("p (r x) -> p r x", r=6)[:, lo:6, :],
                    AF.Relu, bias=cb32[:MID, 1:2], scale=cb32[:MID, 0:1],
                )

            wt_sb = wtpool.tile([ENC, HQ, WTW], F16)
            wtn_sb = wtpool.tile([ENC, HQ, WTW], F16)
            nc.vector.memset(wtn_sb[:, :, 0:4], 0.0)
            nc.vector.memset(wtn_sb[:, :, 4 + W:WTW], 0.0)
            bands = {}
            psums = {}

            # conv2 blocks: two 4-row blocks first (so the band pipeline
            # starts early), then one 8-row block (N=512 matmuls, fewer
            # per-instruction overheads).
            for half, (r0, NR) in enumerate(((0, 4), (4, 4), (8, 8))):
                pse = psmisc.tile([128, 512], F32, tag="ps", name=f"pse{half}")
                for dx in range(3):
                    nc.tensor.matmul(
                        pse[:ENC, :NR * W],
                        cbB[:, dx * ENC:dx * ENC + ENC],
                        comp_sb[:, r0:r0 + NR, dx + 1:dx + 1 + W],
                        start=(dx == 0),
                        stop=False,
                    )
                for dx in range(3):
                    nc.tensor.matmul(
                        pse[:ENC, :NR * W],
                        cbB[:MID, 300 + dx * ENC:300 + dx * ENC + ENC],
                        comp_sb[:MID, r0 + 2:r0 + 2 + NR, dx + 1:dx + 1 + W],
                        start=False,
                        stop=(dx == 2),
                    )
                nc.scalar.activation(
                    wt_sb[:, r0:r0 + NR, 4:4 + W],
                    pse[:ENC, :NR * W].rearrange("p (r x) -> p r x", r=NR),
                    AF.Exp, bias=cb32[:ENC, 3:4], scale=cb32[:ENC, 2:3],
                )
                pss = psmisc.tile([128, 512], F32, tag="ps", name=f"pss{half}")
                nc.tensor.matmul(
                    pss[:4, :NR * W], cbB[:ENC, 600:604],
                    wt_sb[:, r0:r0 + NR, 4:4 + W],
                    start=True, stop=True,
                )
                rec = work.tile([4, NR * W], F32, tag="rec", name=f"rec{half}")
                nc.vector.reciprocal_approx_fast(rec[:], pss[:4, :NR * W])
                rec16 = work.tile([4, NR * W], F16, tag="rec16", name=f"rec16{half}")
                nc.vector.tensor_copy(rec16[:], rec[:])
                psr = psmisc.tile([128, 512], F32, tag="ps", name=f"psr{half}")
                nc.tensor.matmul(psr[:ENC, :NR * W], cbB[:4, 604:704], rec16[:],
                                 start=True, stop=True)
                nc.vector.tensor_tensor(
                    out=wtn_sb[:, r0:r0 + NR, 4:4 + W],
                    in0=wt_sb[:, r0:r0 + NR, 4:4 + W],
                    in1=psr[:ENC, :NR * W].rearrange("p (r x) -> p r x", r=NR),
                    op=mybir.AluOpType.mult,
                )

                # bands for this quarter: shifted transposes -> V -> local_scatter
                for Y in range(r0, r0 + NR):
                    # transpose j emits only its 20-channel j-group by
                    # streaming a sliced identity (cols 20j..20j+20).
                    pv = pstr.tile([XW, ENC], F16, tag="pv", name=f"pv{Y}")
                    for j in range(K_UP):
                        nc.tensor.transpose(
                            pv[:, 20 * j:20 * j + 20],
                            wtn_sb[:, Y, 4 - j:4 - j + XW],
                            cbB[:ENC, 704 + 20 * j:704 + 20 * j + 20],
                        )
                    v_sb = vpool.tile([80, 104], F16, tag="v", name=f"v{Y}")
                    nc.vector.memset(v_sb[64:80, :], 0.0)
                    nc.vector.memset(v_sb[:64, ENC:104], 0.0)
                    nc.scalar.copy(v_sb[:XW, :ENC], pv[:])
                    band = bandpool.tile([80, 1280], F16, tag="band", name=f"band{Y}")
                    nc.gpsimd.local_scatter(
                        out_ap=band[:],
                        data_ap=v_sb[:, :ENC],
                        idxs_ap=sidx_sb[:],
                        channels=80,
                        num_elems=1280,
                        num_idxs=ENC,
                    )
                    bands[Y] = band

            # ---- dense banded-matmul stretch ----
            for s in range(RS):
                group = [(i, s - i) for i in range(K_UP) if 0 <= s - i < HQ]
                if s < HQ:
                    po_t = psout.tile([128, 512], F32, tag="po", name=f"po{s}")
                    psums[s] = po_t
                for cb in range(2):
                    for i, Y in group:
                        nc.tensor.matmul(
                            psums[Y][:, 256 * cb:256 * cb + 256],
                            xT_sb[:, cb, s, :],
                            bands[Y][:XW, 256 * i:256 * i + 256],
                            start=(i == 0 and cb == 0),
                            stop=(i == 4 and cb == 1),
                        )
                if s >= 4:
                    Y = s - 4
                    po = psums.pop(Y)
                    # out_sb is y-major: [p, 2Y+py, cb, (2X+px)]
                    nc.scalar.copy(
                        out_sb[:, 2 * Y:2 * Y + 2, 0, :],
                        po[:, 0:256].rearrange("p (py x) -> p py x", py=2),
                    )
                    nc.vector.tensor_copy(
                        out_sb[:, 2 * Y:2 * Y + 2, 1, :],
                        po[:, 256:512].rearrange("p (py x) -> p py x", py=2),
                    )
                    dma_eng = nc.sync if Y % 2 == 0 else nc.scalar
                    if Y >= 12:
                        # stream the tail row-pairs individually so the last
                        # DMA after the final matmul is as small as possible
                        dma_eng.dma_start(
                            out[:, 2 * Y:2 * Y + 2, :, :].rearrange(
                                "p y c x -> p (y c x)"),
                            out_sb[:, 2 * Y:2 * Y + 2, :, :].rearrange(
                                "p y c x -> p (y c x)"),
                        )
                    elif Y % 2 == 1:
                        Y0 = Y - 1
                        nc.sync.dma_start(
                            out[:, 2 * Y0:2 * Y0 + 4, :, :].rearrange(
                                "p y c x -> p (y c x)"),
                            out_sb[:, 2 * Y0:2 * Y0 + 4, :, :].rearrange(
                                "p y c x -> p (y c x)"),
                        )

    _dedup_ldweights(nc)
    nc.compile()
    return nc


def _dedup_ldweights(nc):
    """Drop Ldweights whose weights AP equals the previously loaded one on
    the PE stream (Tile splits every matmul into Ldweights+Matmult with
    non-self-loading Matmults, so a redundant reload is a pure no-op).

    A redundant Ldweights that carries semaphore waits is still removable:
    its waits are moved onto the immediately following Matmult (same engine
    queue, so every ordering the wait enforced still holds)."""
    def _rows(ins):
        if ins.is_transpose:
            return (0, 128)  # transpose stationary spans the K=100 rows
        tp = ins.tile_position or (0, 0)
        ts = ins.tile_size
        r = ts[0] if ts else 128
        return (tp[0], tp[0] + r)

    removed = 0
    for blk in nc.m.functions[0].blocks:
        insts = blk.instructions
        last = {}  # strip row-offset -> (key, row-range)
        idx = 0
        while idx < len(insts):
            ins = insts[idx]
            if ins.opcode == 'Ldweights':
                strip = (ins.tile_position or (0, 0))[0]
                r0, r1 = _rows(ins)
                key = str(ins.ins[0]) + f"|{ins.is_transpose}|{ins.perf_mode}"
                si = ins.sync_info
                no_upd = si is None or not si.on_update
                prev = last.get(strip)
                if not ins.is_transpose and prev is not None and prev[0] == key \
                        and no_upd and idx + 1 < len(insts) \
                        and insts[idx + 1].opcode == 'Matmult':
                    if si is not None and si.on_wait:
                        nxt = insts[idx + 1]
                        nsi = nxt.sync_info
                        if nsi is None:
                            nxt.sync_info = si
                        else:
                            try:
                                nsi.on_wait.extend(si.on_wait)
                            except Exception:
                                nxt.sync_info = mybir.SyncInfo(
                                    on_wait=list(nsi.on_wait) + list(si.on_wait),
                                    on_update=list(nsi.on_update),
                                )
                    del insts[idx]
                    removed += 1
                    continue
                # this load clobbers any tracked strip whose rows overlap
                for k in list(last):
                    kr = last[k][1]
                    if not (r1 <= kr[0] or kr[1] <= r0):
                        del last[k]
                if not ins.is_transpose:
                    last[strip] = (key, (r0, r1))
            idx += 1
    return removed


def _prep_shared_inputs(w_comp, b_comp, g1, be1, m1, v1, w_enc, b_enc, g2, be2, m2, v2):
    perm = _channel_perm()
    w_comp = np.asarray(w_comp, np.float32).reshape(MID, C)
    wcompT = np.ascontiguousarray(w_comp.T.reshape(2, 128, MID))
    s1 = (np.asarray(g1) / np.sqrt(np.asarray(v1) + EPS)).astype(np.float32)
    t1 = (np.asarray(b_comp) * s1 + np.asarray(be1) - np.asarray(m1) * s1).astype(np.float32)

    w_enc_p = np.asarray(w_enc, np.float32)[perm]          # [100, 64, 3, 3]
    wencT = np.ascontiguousarray(
        w_enc_p.transpose(1, 2, 3, 0).reshape(MID, 9, ENC)
    )  # [ic, tap(dy*3+dx), oc']
    s2f = (np.asarray(g2) / np.sqrt(np.asarray(v2) + EPS)).astype(np.float32)[perm]
    t2f = (np.asarray(b_enc) * (np.asarray(g2) / np.sqrt(np.asarray(v2) + EPS))
           + np.asarray(be2) - np.asarray(m2) * np.asarray(g2) / np.sqrt(np.asarray(v2) + EPS)
           ).astype(np.float32)[perm]

    sub = np.arange(ENC) % 4
    sel4T = np.zeros((ENC, 4), np.float32)
    sel4T[np.arange(ENC), sub] = 1.0
    selbT = np.ascontiguousarray(sel4T.T)

    return {
        "wcompT": wcompT,
        "s1": s1.reshape(MID, 1), "t1": t1.reshape(MID, 1),
        "wencT": wencT,
        "s2": s2f.reshape(ENC, 1), "t2": t2f.reshape(ENC, 1),
        "sel4T": sel4T, "selbT": selbT,
    }


def _scatter_idx():
    """idx[x', j*20 + i*4 + py*2 + px] = i*256 + py*128 + 2*(x'-j) + px,
    or -1 when x' >= 68 or (x'-j) outside [0, W)."""
    idx = np.full((80, ENC), -1, np.int16)
    for xp in range(XW):
        for j in range(K_UP):
            X_ = xp - j
            if not (0 <= X_ < W):
                continue
            for i in range(K_UP):
                for py in range(2):
                    for px in range(2):
                        col = j * 20 + i * 4 + py * 2 + px
                        idx[xp, col] = i * 256 + py * 128 + 2 * X_ + px
    return idx


def _prep_shared_v3(shared):
    # exp() results are stored in fp16; shift logits by a constant (softmax
    # is invariant) to keep exp(x-4) well inside fp16 range.
    t2 = shared["t2"] - 4.0
    wcompT = shared["wcompT"].astype(np.float16)  # [2, 128, MID]
    wencT = shared["wencT"].reshape(MID, 3, 3, ENC)  # [ic, dy, dx, oc']
    wencT2a = np.ascontiguousarray(
        np.concatenate([wencT[:, 0], wencT[:, 1]], axis=0)).astype(np.float16)
    wencT2b = np.ascontiguousarray(wencT[:, 2]).astype(np.float16)

    blobA = np.ascontiguousarray(
        wcompT.transpose(1, 0, 2).reshape(128, 2 * MID))

    blobB = np.zeros((128, 832), np.float16)
    blobB[:, 0:300] = wencT2a.reshape(128, 300)
    blobB[:MID, 300:600] = wencT2b.reshape(MID, 300)
    blobB[:ENC, 600:604] = shared["sel4T"]
    blobB[:4, 604:704] = shared["selbT"]
    blobB[:, 704:832] = np.eye(128, dtype=np.float16)

    blob32 = np.zeros((128, 4), np.float32)
    blob32[:MID, 0] = shared["s1"][:, 0]
    blob32[:MID, 1] = shared["t1"][:, 0]
    blob32[:ENC, 2] = shared["s2"][:, 0]
    blob32[:ENC, 3] = t2[:, 0]

    return {"blobA": blobA, "blobB": blobB, "blob32": blob32,
            "scatidx": _scatter_idx()}


def make_in_maps(X, shared):
    X = np.asarray(X, np.float32)
    in_maps = []
    for core in range(8):
        b, q = divmod(core, 4)
        slab = np.zeros((C, RS, XW), np.float32)
        lo, hi = 16 * q - 2, 16 * q + 18
        slo, shi = max(lo, 0), min(hi, H)
        slab[:, slo - lo:shi - lo, 2:2 + W] = X[b, :, slo:shi, :]
        xs = np.ascontiguousarray(slab.reshape(2, 128, RS, XW).transpose(1, 0, 2, 3))
        xs16 = xs.astype(np.float16)
        # [x', cb, r, c] transposed slab
        xsT = np.ascontiguousarray(xs16.transpose(3, 1, 2, 0))
        in_maps.append({"xslab": xs16, "xslabT": xsT, **shared})
    return in_maps


VERSION = 3


def kernel(X, w_comp, b_comp, bn1_gamma, bn1_beta, bn1_mean, bn1_var,
           w_enc, b_enc, bn2_gamma, bn2_beta, bn2_mean, bn2_var):
    key = ("nc", VERSION)
    if key not in _CACHE:
        _CACHE[key] = _build_program_v3()
    nc = _CACHE[key]

    shared = _prep_shared_inputs(w_comp, b_comp, bn1_gamma, bn1_beta, bn1_mean,
                                 bn1_var, w_enc, b_enc, bn2_gamma, bn2_beta,
                                 bn2_mean, bn2_var)
    shared = _prep_shared_v3(shared)
    in_maps = make_in_maps(X, shared)
    res = bass_utils.run_bass_kernel_spmd(nc, in_maps, core_ids=list(range(8)))

    out = np.zeros((B, C, 2 * H, 2 * W), np.float32)
    for core in range(8):
        b, q = divmod(core, 4)
        o = res.results[core]["out"]  # [128, 32, 2, 128] y-major
        out[b, :, 32 * q:32 * q + 32, :] = o.transpose(2, 0, 1, 3).reshape(C, 32, 128)
    return out


# revision 30
# speedup vs baseline: 1.1746x; 1.1746x over previous
"""CARAFE kernel for Trainium2 (8 NeuronCores, data+H-spatial sharded).

Full inputs in, full output out. Inside: host pre-shards X into per-core
zero-padded slabs (batch x H-quarter), one identical Bass/Tile program runs
SPMD on 8 cores, host gathers the output slabs.

v3: PE warmup chain (probe the cold-clock throttle), split/reordered input
DMA so conv1 starts early, conv2 emitted in 8-row halves (N=512 matmuls),
output laid out y-major for larger DMA chunks, edge-only memsets.
"""

import numpy as np

import concourse.bass as bass
import concourse.mybir as mybir
import concourse.tile as tile
from concourse import bacc, bass_utils

F32 = mybir.dt.float32
F16 = mybir.dt.float16
I16 = mybir.dt.int16
AF = mybir.ActivationFunctionType

SCALE = 2
K_UP = 5
K_ENC = 3
EPS = 1e-5
B, C, H, W = 2, 256, 64, 64
MID = 64
ENC = 100  # (SCALE*K_UP)**2
HQ = H // 4          # 16 input rows per core
RS = HQ + 4          # 20 slab rows (halo 2 each side)
XW = W + 4           # 68 padded width
WTW = W + 8          # 72 padded width for weight rows (phase-2 shifts)

WARMUP = 36          # dummy N=128 matmuls to try to unthrottle the PE clock

_CACHE = {}


def _channel_perm():
    """New enc-channel order: c' = j*20 + i*4 + py*2 + px.
    Original torch order: c = (i*K_UP + j)*4 + py*2 + px."""
    perm = np.zeros(ENC, dtype=np.int64)
    for j in range(K_UP):
        for i in range(K_UP):
            for py in range(SCALE):
                for px in range(SCALE):
                    newc = ((j * K_UP + i) * SCALE + py) * SCALE + px
                    oldc = ((i * K_UP + j) * SCALE + py) * SCALE + px
                    perm[newc] = oldc
    return perm


def _build_program_v3():
    """Banded-matmul reassembly. Per output row-pair Y: 5 shifted PE
    transposes of the normalized softmax weights produce
    V[x'-partition, (j,i,py,px)] (each transpose uses an identity SLICE so
    only its own 20-channel j-group is emitted); a gpsimd local_scatter
    places V into a sparse band matrix band[x', i*256 + py*128 + 2*(x'-j)
    + px]; then 5 fp16 matmuls against the host-pre-transposed X slab
    contract x' and accumulate over the i taps in PSUM."""
    nc = bacc.Bacc("TRN2", target_bir_lowering=False, debug=False)

    xslab = nc.dram_tensor("xslab", [128, 2, RS, XW], F16, kind="ExternalInput").ap()
    xslabT = nc.dram_tensor("xslabT", [XW, 2, RS, 128], F16, kind="ExternalInput").ap()
    # fp16 constants packed into blobs. blobA = conv1 weights (cols 0:128 of
    # the old blob16); blobB cols: [0:300) wencT2a (dx*100+oc), [300:600)
    # wencT2b, [600:604) sel4T, [604:704) selbT, [704:832) eye128
    blobA = nc.dram_tensor("blobA", [128, 128], F16, kind="ExternalInput").ap()
    blobB = nc.dram_tensor("blobB", [128, 832], F16, kind="ExternalInput").ap()
    # blob32 cols: 0 s1, 1 t1 (rows :64); 2 s2, 3 t2 (rows :100)
    blob32 = nc.dram_tensor("blob32", [128, 4], F32, kind="ExternalInput").ap()
    scatidx = nc.dram_tensor("scatidx", [80, ENC], I16, kind="ExternalInput").ap()
    # output y-major so per-partition DMA chunks are (2*2W) contiguous
    out = nc.dram_tensor("out", [128, 2 * HQ, 2, 2 * W], F16, kind="ExternalOutput").ap()

    with tile.TileContext(nc) as tc:
        with (
            tc.tile_pool(name="consts", bufs=1) as consts,
            tc.tile_pool(name="xpool", bufs=1) as xpool,
            tc.tile_pool(name="comp", bufs=1) as comppool,
            tc.tile_pool(name="wt", bufs=1) as wtpool,
            tc.tile_pool(name="vsb", bufs=4) as vpool,
            tc.tile_pool(name="band", bufs=17) as bandpool,
            tc.tile_pool(name="work", bufs=2) as work,
            tc.tile_pool(name="psout", bufs=5, space="PSUM") as psout,
            tc.tile_pool(name="psmisc", bufs=2, space="PSUM") as psmisc,
            tc.tile_pool(name="pstr", bufs=1, space="PSUM") as pstr,
        ):
            # ---- input DMAs, in consumption order (single SP ring — the
            # Activation HWDGE ring measurably hurts: its triggers
            # serialize against ACT compute) ----
            cbA = consts.tile([128, 128], F16)
            nc.sync.dma_start(cbA[:], blobA)
            xA_sb = xpool.tile([128, 2, 13, XW], F16)
            # conv1 tile 0 needs slab rows 1..6 of both c-blocks: ship
            # those rows first so the first matmul pair can start sooner.
            nc.sync.dma_start(xA_sb[:, 0, 0:7], xslab[:, 0, 0:7, :])
            nc.sync.dma_start(xA_sb[:, 1, 0:7], xslab[:, 1, 0:7, :])
            cb32 = consts.tile([128, 4], F32)
            nc.sync.dma_start(cb32[:], blob32)
            nc.sync.dma_start(xA_sb[:, 0, 7:13], xslab[:, 0, 7:13, :])
            nc.sync.dma_start(xA_sb[:, 1, 7:13], xslab[:, 1, 7:13, :])
            cbB = consts.tile([128, 832], F16)
            nc.sync.dma_start(cbB[:], blobB)
            xB_sb = xpool.tile([128, 2, RS - 13, XW], F16)
            nc.sync.dma_start(xB_sb[:, 0], xslab[:, 0, 13:RS, :])
            nc.sync.dma_start(xB_sb[:, 1], xslab[:, 1, 13:RS, :])
            sidx_sb = consts.tile([80, ENC], I16)
            nc.sync.dma_start(sidx_sb[:], scatidx)
            xT_sb = xpool.tile([XW, 2, RS, 128], F16)
            nc.sync.dma_start(xT_sb[:, 0], xslabT[:, 0])
            nc.sync.dma_start(xT_sb[:, 1], xslabT[:, 1])
            out_sb = xpool.tile([128, 2 * HQ, 2, 2 * W], F16)

            # ---- conv1 + bn1 + relu -> comp [128, 18, 68] fp16 ----
            # rows 0:64 hold comp[r]; rows 64:128 hold comp[r+1] so the
            # enc conv can contract two dy taps per K=128 matmul.
            comp_sb = comppool.tile([128, RS - 2, XW], F16)
            # only the column halo (0:2, 66:68) is never written
            nc.vector.memset(comp_sb[:, :, 0:2], 0.0)
            nc.vector.memset(comp_sb[:, :, 2 + W:XW], 0.0)
            for rt in range(3):
                r0 = rt * 6
                xt, xoff = (xA_sb, 0) if rt < 2 else (xB_sb, 13)
                ps = psmisc.tile([128, 512], F32, tag="ps")
                for cb in range(2):
                    nc.tensor.matmul(
                        ps[:MID, :6 * W],
                        cbA[:, cb * MID:cb * MID + MID],
                        xt[:, cb, 1 + r0 - xoff:1 + r0 - xoff + 6, 2:2 + W],
                        start=(cb == 0),
                        stop=(cb == 1),
                    )
                nc.scalar.activation(
                    comp_sb[:MID, r0:r0 + 6, 2:2 + W],
                    ps[:MID, :6 * W].rearrange("p (r x) -> p r x", r=6),
                    AF.Relu, bias=cb32[:MID, 1:2], scale=cb32[:MID, 0:1],
                )
                lo = 1 if rt == 0 else 0
                nc.scalar.activation(
                    comp_sb[MID:128, r0 - 1 + lo:r0 + 5, 2:2 + W],
                    ps[:MID, :6 * W].rearrange("p (r x) -> p r x", r=6)[:, lo:6, :],
                    AF.Relu, bias=cb32[:MID, 1:2], scale=cb32[:MID, 0:1],
                )

            wt_sb = wtpool.tile([ENC, HQ, WTW], F16)
            wtn_sb = wtpool.tile([ENC, HQ, WTW], F16)
            nc.vector.memset(wtn_sb[:, :, 0:4], 0.0)
            nc.vector.memset(wtn_sb[:, :, 4 + W:WTW], 0.0)
            bands = {}
            psums = {}

            # conv2 blocks: two 4-row blocks first (so the band pipeline
            # starts early), then one 8-row block (N=512 matmuls, fewer
            # per-instruction overheads).
            for half, (r0, NR) in enumerate(((0, 4), (4, 4), (8, 8))):
                pse = psmisc.tile([128, 512], F32, tag="ps", name=f"pse{half}")
                for dx in range(3):
                    nc.tensor.matmul(
                        pse[:ENC, :NR * W],
                        cbB[:, dx * ENC:dx * ENC + ENC],
                        comp_sb[:, r0:r0 + NR, dx + 1:dx + 1 + W],
                        start=(dx == 0),
                        stop=False,
                    )
                for dx in range(3):
                    nc.tensor.matmul(
                        pse[:ENC, :NR * W],
                        cbB[:MID, 300 + dx * ENC:300 + dx * ENC + ENC],
                        comp_sb[:MID, r0 + 2:r0 + 2 + NR, dx + 1:dx + 1 + W],
                        start=False,
                        stop=(dx == 2),
                    )
                nc.scalar.activation(
                    wt_sb[:, r0:r0 + NR, 4:4 + W],
                    pse[:ENC, :NR * W].rearrange("p (r x) -> p r x", r=NR),
                    AF.Exp, bias=cb32[:ENC, 3:4], scale=cb32[:ENC, 2:3],
                )
                pss = psmisc.tile([128, 512], F32, tag="ps", name=f"pss{half}")
                nc.tensor.matmul(
                    pss[:4, :NR * W], cbB[:ENC, 600:604],
                    wt_sb[:, r0:r0 + NR, 4:4 + W],
                    start=True, stop=True,
                )
                rec = work.tile([4, NR * W], F32, tag="rec", name=f"rec{half}")
                nc.vector.reciprocal_approx_fast(rec[:], pss[:4, :NR * W])
                rec16 = work.tile([4, NR * W], F16, tag="rec16", name=f"rec16{half}")
                nc.vector.tensor_copy(rec16[:], rec[:])
                psr = psmisc.tile([128, 512], F32, tag="ps", name=f"psr{half}")
                nc.tensor.matmul(psr[:ENC, :NR * W], cbB[:4, 604:704], rec16[:],
                                 start=True, stop=True)
                nc.vector.tensor_tensor(
                    out=wtn_sb[:, r0:r0 + NR, 4:4 + W],
                    in0=wt_sb[:, r0:r0 + NR, 4:4 + W],
                    in1=psr[:ENC, :NR * W].rearrange("p (r x) -> p r x", r=NR),
                    op=mybir.AluOpType.mult,
                )

                # bands for this quarter: shifted transposes -> V -> local_scatter
                for Y in range(r0, r0 + NR):
                    # transpose j emits only its 20-channel j-group by
                    # streaming a sliced identity (cols 20j..20j+20).
                    pv = pstr.tile([XW, ENC], F16, tag="pv", name=f"pv{Y}")
                    for j in range(K_UP):
                        nc.tensor.transpose(
                            pv[:, 20 * j:20 * j + 20],
                            wtn_sb[:, Y, 4 - j:4 - j + XW],
                            cbB[:ENC, 704 + 20 * j:704 + 20 * j + 20],
                        )
                    v_sb = vpool.tile([80, 104], F16, tag="v", name=f"v{Y}")
                    nc.vector.memset(v_sb[64:80, :], 0.0)
                    nc.vector.memset(v_sb[:64, ENC:104], 0.0)
                    nc.scalar.copy(v_sb[:XW, :ENC], pv[:])
                    band = bandpool.tile([80, 1280], F16, tag="band", name=f"band{Y}")
                    nc.gpsimd.local_scatter(
                        out_ap=band[:],
                        data_ap=v_sb[:, :ENC],
                        idxs_ap=sidx_sb[:],
                        channels=80,
                        num_elems=1280,
                        num_idxs=ENC,
                    )
                    bands[Y] = band

            # ---- dense banded-matmul stretch ----
            for s in range(RS):
                group = [(i, s - i) for i in range(K_UP) if 0 <= s - i < HQ]
                if s < HQ:
                    po_t = psout.tile([128, 512], F32, tag="po", name=f"po{s}")
                    psums[s] = po_t
                for cb in range(2):
                    for i, Y in group:
                        nc.tensor.matmul(
                            psums[Y][:, 256 * cb:256 * cb + 256],
                            xT_sb[:, cb, s, :],
                            bands[Y][:XW, 256 * i:256 * i + 256],
                            start=(i == 0 and cb == 0),
                            stop=(i == 4 and cb == 1),
                        )
                if s >= 4:
                    Y = s - 4
                    po = psums.pop(Y)
                    # out_sb is y-major: [p, 2Y+py, cb, (2X+px)]
                    nc.scalar.copy(
                        out_sb[:, 2 * Y:2 * Y + 2, 0, :],
                        po[:, 0:256].rearrange("p (py x) -> p py x", py=2),
                    )
                    nc.vector.tensor_copy(
                        out_sb[:, 2 * Y:2 * Y + 2, 1, :],
                        po[:, 256:512].rearrange("p (py x) -> p py x", py=2),
                    )
                    if Y >= 12:
                        # stream the tail row-pairs individually so the last
                        # DMA after the final matmul is as small as possible
                        nc.sync.dma_start(
                            out[:, 2 * Y:2 * Y + 2, :, :].rearrange(
                                "p y c x -> p (y c x)"),
                            out_sb[:, 2 * Y:2 * Y + 2, :, :].rearrange(
                                "p y c x -> p (y c x)"),
                        )
                    elif Y % 2 == 1:
                        Y0 = Y - 1
                        nc.sync.dma_start(
                            out[:, 2 * Y0:2 * Y0 + 4, :, :].rearrange(
                                "p y c x -> p (y c x)"),
                            out_sb[:, 2 * Y0:2 * Y0 + 4, :, :].rearrange(
                                "p y c x -> p (y c x)"),
                        )

    _dedup_ldweights(nc)
    nc.compile()
    return nc


def _dedup_ldweights(nc):
    """Drop Ldweights whose weights AP equals the previously loaded one on
    the PE stream (Tile splits every matmul into Ldweights+Matmult with
    non-self-loading Matmults, so a redundant reload is a pure no-op).

    A redundant Ldweights that carries semaphore waits is still removable:
    its waits are moved onto the immediately following Matmult (same engine
    queue, so every ordering the wait enforced still holds)."""
    def _rows(ins):
        if ins.is_transpose:
            return (0, 128)  # transpose stationary spans the K=100 rows
        tp = ins.tile_position or (0, 0)
        ts = ins.tile_size
        r = ts[0] if ts else 128
        return (tp[0], tp[0] + r)

    removed = 0
    for blk in nc.m.functions[0].blocks:
        insts = blk.instructions
        last = {}  # strip row-offset -> (key, row-range)
        idx = 0
        while idx < len(insts):
            ins = insts[idx]
            if ins.opcode == 'Ldweights':
                strip = (ins.tile_position or (0, 0))[0]
                r0, r1 = _rows(ins)
                key = str(ins.ins[0]) + f"|{ins.is_transpose}|{ins.perf_mode}"
                si = ins.sync_info
                no_upd = si is None or not si.on_update
                prev = last.get(strip)
                if not ins.is_transpose and prev is not None and prev[0] == key \
                        and no_upd and idx + 1 < len(insts) \
                        and insts[idx + 1].opcode == 'Matmult':
                    if si is not None and si.on_wait:
                        nxt = insts[idx + 1]
                        nsi = nxt.sync_info
                        if nsi is None:
                            nxt.sync_info = si
                        else:
                            try:
                                nsi.on_wait.extend(si.on_wait)
                            except Exception:
                                nxt.sync_info = mybir.SyncInfo(
                                    on_wait=list(nsi.on_wait) + list(si.on_wait),
                                    on_update=list(nsi.on_update),
                                )
                    del insts[idx]
                    removed += 1
                    continue
                # this load clobbers any tracked strip whose rows overlap
                for k in list(last):
                    kr = last[k][1]
                    if not (r1 <= kr[0] or kr[1] <= r0):
                        del last[k]
                if not ins.is_transpose:
                    last[strip] = (key, (r0, r1))
            idx += 1
    return removed


def _prep_shared_inputs(w_comp, b_comp, g1, be1, m1, v1, w_enc, b_enc, g2, be2, m2, v2):
    perm = _channel_perm()
    w_comp = np.asarray(w_comp, np.float32).reshape(MID, C)
    wcompT = np.ascontiguousarray(w_comp.T.reshape(2, 128, MID))
    s1 = (np.asarray(g1) / np.sqrt(np.asarray(v1) + EPS)).astype(np.float32)
    t1 = (np.asarray(b_comp) * s1 + np.asarray(be1) - np.asarray(m1) * s1).astype(np.float32)

    w_enc_p = np.asarray(w_enc, np.float32)[perm]          # [100, 64, 3, 3]
    wencT = np.ascontiguousarray(
        w_enc_p.transpose(1, 2, 3, 0).reshape(MID, 9, ENC)
    )  # [ic, tap(dy*3+dx), oc']
    s2f = (np.asarray(g2) / np.sqrt(np.asarray(v2) + EPS)).astype(np.float32)[perm]
    t2f = (np.asarray(b_enc) * (np.asarray(g2) / np.sqrt(np.asarray(v2) + EPS))
           + np.asarray(be2) - np.asarray(m2) * np.asarray(g2) / np.sqrt(np.asarray(v2) + EPS)
           ).astype(np.float32)[perm]

    sub = np.arange(ENC) % 4
    sel4T = np.zeros((ENC, 4), np.float32)
    sel4T[np.arange(ENC), sub] = 1.0
    selbT = np.ascontiguousarray(sel4T.T)

    return {
        "wcompT": wcompT,
        "s1": s1.reshape(MID, 1), "t1": t1.reshape(MID, 1),
        "wencT": wencT,
        "s2": s2f.reshape(ENC, 1), "t2": t2f.reshape(ENC, 1),
        "sel4T": sel4T, "selbT": selbT,
    }


def _scatter_idx():
    """idx[x', j*20 + i*4 + py*2 + px] = i*256 + py*128 + 2*(x'-j) + px,
    or -1 when x' >= 68 or (x'-j) outside [0, W)."""
    idx = np.full((80, ENC), -1, np.int16)
    for xp in range(XW):
        for j in range(K_UP):
            X_ = xp - j
            if not (0 <= X_ < W):
                continue
            for i in range(K_UP):
                for py in range(2):
                    for px in range(2):
                        col = j * 20 + i * 4 + py * 2 + px
                        idx[xp, col] = i * 256 + py * 128 + 2 * X_ + px
    return idx


def _prep_shared_v3(shared):
    # exp() results are stored in fp16; shift logits by a constant (softmax
    # is invariant) to keep exp(x-4) well inside fp16 range.
    t2 = shared["t2"] - 4.0
    wcompT = shared["wcompT"].astype(np.float16)  # [2, 128, MID]
    wencT = shared["wencT"].reshape(MID, 3, 3, ENC)  # [ic, dy, dx, oc']
    wencT2a = np.ascontiguousarray(
        np.concatenate([wencT[:, 0], wencT[:, 1]], axis=0)).astype(np.float16)
    wencT2b = np.ascontiguousarray(wencT[:, 2]).astype(np.float16)

    blobA = np.ascontiguousarray(
        wcompT.transpose(1, 0, 2).reshape(128, 2 * MID))

    blobB = np.zeros((128, 832), np.float16)
    blobB[:, 0:300] = wencT2a.reshape(128, 300)
    blobB[:MID, 300:600] = wencT2b.reshape(MID, 300)
    blobB[:ENC, 600:604] = shared["sel4T"]
    blobB[:4, 604:704] = shared["selbT"]
    blobB[:, 704:832] = np.eye(128, dtype=np.float16)

    blob32 = np.zeros((128, 4), np.float32)
    blob32[:MID, 0] = shared["s1"][:, 0]
    blob32[:MID, 1] = shared["t1"][:, 0]
    blob32[:ENC, 2] = shared["s2"][:, 0]
    blob32[:ENC, 3] = t2[:, 0]

    return {"blobA": blobA, "blobB": blobB, "blob32": blob32,
            "scatidx": _scatter_idx()}


def make_in_maps(X, shared):
    X = np.asarray(X, np.float32)
    in_maps = []
    for core in range(8):
        b, q = divmod(core, 4)
        slab = np.zeros((C, RS, XW), np.float32)
        lo, hi = 16 * q - 2, 16 * q + 18
        slo, shi = max(lo, 0), min(hi, H)
        slab[:, slo - lo:shi - lo, 2:2 + W] = X[b, :, slo:shi, :]
        xs = np.ascontiguousarray(slab.reshape(2, 128, RS, XW).transpose(1, 0, 2, 3))
        xs16 = xs.astype(np.float16)
        # [x', cb, r, c] transposed slab
        xsT = np.ascontiguousarray(xs16.transpose(3, 1, 2, 0))
        in_maps.append({"xslab": xs16, "xslabT": xsT, **shared})
    return in_maps


VERSION = 3


def kernel(X, w_comp, b_comp, bn1_gamma, bn1_beta, bn1_mean, bn1_var,
           w_enc, b_enc, bn2_gamma, bn2_beta, bn2_mean, bn2_var):
    key = ("nc", VERSION)
    if key not in _CACHE:
        _CACHE[key] = _build_program_v3()
    nc = _CACHE[key]

    shared = _prep_shared_inputs(w_comp, b_comp, bn1_gamma, bn1_beta, bn1_mean,
                                 bn1_var, w_enc, b_enc, bn2_gamma, bn2_beta,
                                 bn2_mean, bn2_var)
    shared = _prep_shared_v3(shared)
    in_maps = make_in_maps(X, shared)
    res = bass_utils.run_bass_kernel_spmd(nc, in_maps, core_ids=list(range(8)))

    out = np.zeros((B, C, 2 * H, 2 * W), np.float32)
    for core in range(8):
        b, q = divmod(core, 4)
        o = res.results[core]["out"]  # [128, 32, 2, 128] y-major
        out[b, :, 32 * q:32 * q + 32, :] = o.transpose(2, 0, 1, 3).reshape(C, 32, 128)
    return out
